# revision 7
# baseline (speedup 1.0000x reference)
"""TRN2 Bass kernel: batch-invariant full attention.

Problem: out = softmax(Q K^T / sqrt(64)) V with Q,K,V f32 [4, 16, 2048, 64].
Sharding: the 64 (batch, head) pairs are split 8 ways across the 8
NeuronCores (8 pairs per core); attention is independent per pair.

Per-core design (per pair), v2 — engine-balanced:
  - Sequence indices are permuted as s = p*T + t (T=16) so every DMA is
    contiguous per partition; consistent between K and V and undone by
    construction on the output path.
  - Inputs are cast f32->fp16 during the load DMA (SWDGE cast); the
    1/sqrt(d) scale is folded into the exp, so Q/K/V loads are plain casts.
  - Q^T / K^T tiles are built by the DMA xbar transpose (HWDGE,
    dma_start(..., transpose=True)) as [128, m, 128] fp16 with row-pair
    interleave: tile 2m's d on partitions 0-63, tile 2m+1's on 64-127.
    No TensorE transposes, no PSUM->SBUF copies.
  - QK runs as concurrent row-group pairs (tile_position from
    base_partition 0/64): per (m, cross) step two N=1024 matmuls produce
    S^T[k-tile, all-even-q] and S^T[k-tile', all-odd-q] in fp16 PSUM
    (1 bank each). A partition-swapped copy (kt2s) covers parity cross
    terms.
  - exp is SPLIT across engines: half the tiles on ScalarE (exact
    e^(sc*x) activation), half on the DVE as a one-instruction base-2
    Schraudolph bitcast: i16 = x*(1024*sc*log2 e) + KOFF, reinterpreted
    as fp16 (~+-3.5% on those weights; softmax denominator uses the same
    approximated values so the bias largely cancels).
  - PV is FLIPPED: stationary = [V | 1] fp16 (65 cols, loaded once per
    k-tile), moving = e[k-tile, q] (N=512). Accumulates out^T=[O; denom]
    [65, 512] per q-quarter directly in PSUM (4 banks per pair). No
    per-k-tile LDWEIGHTS of e => no weight-load bottleneck.
  - Epilogue: 4 strided DVE copies assemble out^T [65, 2048] fp16 in
    natural q order; one xbar DMA transposes the V-part back to q-major
    [128, 16, 64]; the denom row is re-tiled by a tiny DMA; one DVE
    reciprocal + per-t tensor_scalar multiplies produce f32 output;
    contiguous DMA out.
"""
import functools
from contextlib import ExitStack

import numpy as np

import concourse.mybir as mybir
import concourse.tile as tile
from concourse import bacc
from concourse.bass_utils import run_bass_kernel_spmd

F32 = mybir.dt.float32
F16 = mybir.dt.float16
I16 = mybir.dt.int16
EXP = mybir.ActivationFunctionType.Exp
MULT = mybir.AluOpType.mult
ADD = mybir.AluOpType.add

B, H, S, D = 4, 16, 2048, 64
N_CORES = 8
NBH = B * H // N_CORES  # 8 (b,h) pairs per core

SC = 1.0 / 8.0  # 1/sqrt(D)
# DVE base-2 bitcast exp: i16 = round-ish(z*1024*sc*log2(e) + KOFF), viewed
# as fp16. KOFF = 15*1024 (bias) - 36.2 (minimax ratio offset) + 0.5
# (truncation compensation).
KSC = float(1024.0 * SC * np.log2(np.e))
KOFF = float(15 * 1024 - 36.2 + 0.5)


def build_attention(nbh=NBH, S=S, D=D):
    assert D == 64
    T = S // 128  # 16 s-tiles of 128
    M = T // 2  # 8 tile pairs

    nc = bacc.Bacc("TRN2", target_bir_lowering=False, debug=False)
    q = nc.dram_tensor("q", [nbh, S, D], F32, kind="ExternalInput").ap()
    k = nc.dram_tensor("k", [nbh, S, D], F32, kind="ExternalInput").ap()
    v = nc.dram_tensor("v", [nbh, S, D], F32, kind="ExternalInput").ap()
    o = nc.dram_tensor("o", [nbh, S, D], F32, kind="ExternalOutput").ap()

    with tile.TileContext(nc) as tc, ExitStack() as ctx:
        ld = ctx.enter_context(tc.tile_pool(name="ld", bufs=2))
        tp = ctx.enter_context(tc.tile_pool(name="tp", bufs=2))
        ep = ctx.enter_context(tc.tile_pool(name="ep", bufs=6))
        ot = ctx.enter_context(tc.tile_pool(name="ot", bufs=2))
        of = ctx.enter_context(tc.tile_pool(name="of", bufs=2))
        pp_s = ctx.enter_context(tc.tile_pool(name="pp_s", bufs=2, space="PSUM"))
        pp_o = ctx.enter_context(tc.tile_pool(name="pp_o", bufs=1, space="PSUM"))

        for bh in range(nbh):
            # ---- cast-DMA loads (f32 HBM -> fp16 SBUF, s = p*T + t) ----
            q16 = ld.tile([128, T, D], F16, tag="q16")
            k16 = ld.tile([128, T, D], F16, tag="k16")
            vaug = ld.tile([128, T, D + 1], F16, tag="vaug")
            nc.gpsimd.dma_start(
                out=q16, in_=q[bh].rearrange("(p t) d -> p t d", p=128)
            )
            nc.gpsimd.dma_start(
                out=k16, in_=k[bh].rearrange("(p t) d -> p t d", p=128)
            )
            nc.gpsimd.dma_start(
                out=vaug[:, :, 0:D], in_=v[bh].rearrange("(p t) d -> p t d", p=128)
            )
            nc.gpsimd.memset(vaug[:, :, D : D + 1], 1.0)

            # ---- xbar transposes: qt2/kt2 [128, M, 128] interleaved ----
            # qt2[0:64, m, j] = Q^T[d, tile 2m, col j]  (q = j*T + 2m)
            # qt2[64:128, m, j] = Q^T[d, tile 2m+1, col j]
            qt2 = tp.tile([128, M, 128], F16, tag="qt2")
            kt2 = tp.tile([128, M, 128], F16, tag="kt2")
            kt2s = tp.tile([128, M, 128], F16, tag="kt2s")
            for m in range(M):
                nc.sync.dma_start(
                    out=qt2[:, m, :], in_=q16[:, 2 * m : 2 * m + 2, :], transpose=True
                )
                nc.sync.dma_start(
                    out=kt2[:, m, :], in_=k16[:, 2 * m : 2 * m + 2, :], transpose=True
                )
                # partition-swapped copy for the parity cross terms
                nc.gpsimd.dma_start(out=kt2s[0:64, m, :], in_=kt2[64:128, m, :])
                nc.gpsimd.dma_start(out=kt2s[64:128, m, :], in_=kt2[0:64, m, :])

            qt2f = qt2.rearrange("p m j -> p (m j)")
            kt2f = kt2.rearrange("p m j -> p (m j)")
            kt2sf = kt2s.rearrange("p m j -> p (m j)")

            # ---- QK -> exp -> PV ----
            # poT bank j = 2*half + c accumulates [O; denom]^T [65, 512] for
            # q-columns (parity=half, 512-chunk c) over all 16 k-tiles.
            poT = pp_o.tile([65, 4, 512], F32, tag="poT", name=f"poT{bh}")
            for step in range(2 * M):
                m, cross = step // 2, step % 2
                kkf = kt2sf if cross else kt2f
                for half in (0, 1):
                    kb = 2 * m + (cross if half == 0 else 1 - cross)
                    lo, hi = 64 * half, 64 * (half + 1)
                    ps = pp_s.tile(
                        [128, 1024], F32, tag="ps", name=f"ps{bh}_{step}_{half}"
                    )
                    # two N=512 chunks: each matmul output must fit one
                    # 2KB PSUM bank (f32)
                    for c2 in (0, 1):
                        nc.tensor.matmul(
                            out=ps[:, 512 * c2 : 512 * (c2 + 1)],
                            lhsT=kkf[lo:hi, 128 * m : 128 * (m + 1)],
                            rhs=qt2f[lo:hi, 512 * c2 : 512 * (c2 + 1)],
                            start=True,
                            stop=True,
                        )
                    e = ep.tile([128, 1024], F16, tag="e")
                    if (step + half) % 2 == 0:
                        nc.scalar.activation(out=e, in_=ps, func=EXP, scale=SC)
                    else:
                        nc.vector.tensor_scalar(
                            out=e.bitcast(I16),
                            in0=ps,
                            scalar1=KSC,
                            scalar2=KOFF,
                            op0=MULT,
                            op1=ADD,
                        )
                    for c in (0, 1):
                        nc.tensor.matmul(
                            out=poT[:, 2 * half + c, :],
                            lhsT=vaug[:, kb, :],
                            rhs=e[:, 512 * c : 512 * (c + 1)],
                            start=step == 0,
                            stop=step == 2 * M - 1,
                        )

            # ---- epilogue ----
            # Assemble out^T [65, 2048] fp16 with natural q columns:
            # poT[:, 2*half+c, mq*128 + j] corresponds to q = j*16 + 8*c +
            # 2*mq + half.
            # outT col C = t*128 + j holds q = j*16 + t, so the xbar (which
            # writes logical row r to partition r%128, free slot r//128)
            # lands q exactly at [p=q//16, t=q%16]. Padded to 80 partitions
            # (xbar needs %16) so the denom row rides the same transpose.
            outT = ot.tile([80, S], F16, tag="outT")
            outT_r = outT[0:65].rearrange("p (t j) -> p t j", t=16)
            for j4 in range(4):
                half, c = j4 // 2, j4 % 2
                base = 8 * c + half
                nc.vector.tensor_copy(
                    out=outT_r[:, base : base + 7 : 2, :],
                    in_=poT[:, j4, :].rearrange("p (mq j) -> p mq j", mq=4),
                )
            out16x = of.tile([128, T, 80], F16, tag="out16x")
            nc.sync.dma_start(out=out16x, in_=outT, transpose=True)
            rcp = of.tile([128, T], F32, tag="rcp")
            nc.vector.reciprocal(out=rcp, in_=out16x[:, :, 64])
            outf = of.tile([128, T, D], F32, tag="outf")
            for t in range(T):
                nc.vector.tensor_scalar_mul(
                    out=outf[:, t, :], in0=out16x[:, t, 0:D], scalar1=rcp[:, t : t + 1]
                )
            nc.gpsimd.dma_start(
                out=o[bh].rearrange("(p t) d -> p t d", p=128), in_=outf
            )
    nc.compile()
    return nc


@functools.lru_cache(maxsize=1)
def _built():
    return build_attention()


def run(query, key, value, trace=False):
    """Shard (b,h) pairs 8 ways, run on cores 0-7, gather. Returns
    (out [B,H,S,D] f32, BassKernelResults)."""
    nc = _built()
    qf = np.ascontiguousarray(np.asarray(query, dtype=np.float32).reshape(B * H, S, D))
    kf = np.ascontiguousarray(np.asarray(key, dtype=np.float32).reshape(B * H, S, D))
    vf = np.ascontiguousarray(np.asarray(value, dtype=np.float32).reshape(B * H, S, D))
    in_maps = []
    for c in range(N_CORES):
        sl = slice(c * NBH, (c + 1) * NBH)
        in_maps.append(
            {
                "q": np.ascontiguousarray(qf[sl]),
                "k": np.ascontiguousarray(kf[sl]),
                "v": np.ascontiguousarray(vf[sl]),
            }
        )
    res = None
    last_err = None
    for attempt in range(3):
        try:
            res = run_bass_kernel_spmd(
                nc, in_maps, core_ids=list(range(N_CORES)), trace=trace
            )
            break
        except Exception as e:  # transient device wedge: retry
            last_err = e
            import time as _time

            _time.sleep(5 * (attempt + 1))
    if res is None:
        raise last_err
    out = np.concatenate([res.results[c]["o"] for c in range(N_CORES)], axis=0)
    return out.reshape(B, H, S, D).astype(np.float32), res


def kernel(query, key, value):
    out, _ = run(query, key, value)
    return out


# revision 11
# speedup vs baseline: 2.0338x; 2.0338x over previous
"""TRN2 Bass kernel: batch-invariant full attention.

Problem: out = softmax(Q K^T / sqrt(64)) V with Q,K,V f32 [4, 16, 2048, 64].
Sharding: the 64 (batch, head) pairs are split 8 ways across the 8
NeuronCores (8 pairs per core); attention is independent per pair.

Per-core design (per pair), v2 — engine-balanced:
  - Sequence indices are permuted as s = p*T + t (T=16) so every DMA is
    contiguous per partition; consistent between K and V and undone by
    construction on the output path.
  - Inputs are cast f32->fp16 during the load DMA (SWDGE cast); the
    1/sqrt(d) scale is folded into the exp, so Q/K/V loads are plain casts.
  - Q^T / K^T tiles are built by the DMA xbar transpose (HWDGE,
    dma_start(..., transpose=True)) as [128, m, 128] fp16 with row-pair
    interleave: tile 2m's d on partitions 0-63, tile 2m+1's on 64-127.
    No TensorE transposes, no PSUM->SBUF copies.
  - QK runs as concurrent row-group pairs (tile_position from
    base_partition 0/64): per (m, cross) step two N=1024 matmuls produce
    S^T[k-tile, all-even-q] and S^T[k-tile', all-odd-q] in fp16 PSUM
    (1 bank each). A partition-swapped copy (kt2s) covers parity cross
    terms.
  - exp is SPLIT across engines: half the tiles on ScalarE (exact
    e^(sc*x) activation), half on the DVE as a one-instruction base-2
    Schraudolph bitcast: i16 = x*(1024*sc*log2 e) + KOFF, reinterpreted
    as fp16 (~+-3.5% on those weights; softmax denominator uses the same
    approximated values so the bias largely cancels).
  - PV is FLIPPED: stationary = [V | 1] fp16 (65 cols, loaded once per
    k-tile), moving = e[k-tile, q] (N=512). Accumulates out^T=[O; denom]
    [65, 512] per q-quarter directly in PSUM (4 banks per pair). No
    per-k-tile LDWEIGHTS of e => no weight-load bottleneck.
  - Epilogue: 4 strided DVE copies assemble out^T [65, 2048] fp16 in
    natural q order; one xbar DMA transposes the V-part back to q-major
    [128, 16, 64]; the denom row is re-tiled by a tiny DMA; one DVE
    reciprocal + per-t tensor_scalar multiplies produce f32 output;
    contiguous DMA out.
"""
import functools
from contextlib import ExitStack

import numpy as np

import concourse.mybir as mybir
import concourse.tile as tile
from concourse import bacc
from concourse.bass_utils import run_bass_kernel_spmd

F32 = mybir.dt.float32
F16 = mybir.dt.float16
I16 = mybir.dt.int16
EXP = mybir.ActivationFunctionType.Exp
MULT = mybir.AluOpType.mult
ADD = mybir.AluOpType.add

B, H, S, D = 4, 16, 2048, 64
N_CORES = 8
NBH = B * H // N_CORES  # 8 (b,h) pairs per core

SC = 1.0 / 8.0  # 1/sqrt(D)
# DVE base-2 bitcast exp: i16 = round-ish(z*1024*sc*log2(e) + KOFF), viewed
# as fp16. KOFF = 15*1024 (bias) - 36.2 (minimax ratio offset) + 0.5
# (truncation compensation).
KSC = float(1024.0 * SC * np.log2(np.e))
KOFF = float(15 * 1024 - 36.2 + 0.5)


def build_attention(nbh=NBH, S=S, D=D):
    assert D == 64
    T = S // 128  # 16 s-tiles of 128
    M = T // 2  # 8 tile pairs

    nc = bacc.Bacc("TRN2", target_bir_lowering=False, debug=False)
    q = nc.dram_tensor("q", [nbh, S, D], F32, kind="ExternalInput").ap()
    k = nc.dram_tensor("k", [nbh, S, D], F32, kind="ExternalInput").ap()
    v = nc.dram_tensor("v", [nbh, S, D], F32, kind="ExternalInput").ap()
    o = nc.dram_tensor("o", [nbh, S, D], F32, kind="ExternalOutput").ap()

    with tile.TileContext(nc) as tc, ExitStack() as ctx:
        ld = ctx.enter_context(tc.tile_pool(name="ld", bufs=2))
        tp = ctx.enter_context(tc.tile_pool(name="tp", bufs=2))
        ep = ctx.enter_context(tc.tile_pool(name="ep", bufs=10))
        ot = ctx.enter_context(tc.tile_pool(name="ot", bufs=2))
        of = ctx.enter_context(tc.tile_pool(name="of", bufs=2))
        pp_s = ctx.enter_context(tc.tile_pool(name="pp_s", bufs=4, space="PSUM"))
        pp_o = ctx.enter_context(tc.tile_pool(name="pp_o", bufs=1, space="PSUM"))

        for bh in range(nbh):
            # ---- cast-DMA loads (f32 HBM -> fp16 SBUF, s = p*T + t) ----
            q16 = ld.tile([128, T, D], F16, tag="q16")
            k16 = ld.tile([128, T, D], F16, tag="k16")
            vaug = ld.tile([128, T, D + 1], F16, tag="vaug")
            nc.gpsimd.dma_start(
                out=q16, in_=q[bh].rearrange("(p t) d -> p t d", p=128)
            )
            nc.gpsimd.dma_start(
                out=k16, in_=k[bh].rearrange("(p t) d -> p t d", p=128)
            )
            nc.gpsimd.dma_start(
                out=vaug[:, :, 0:D], in_=v[bh].rearrange("(p t) d -> p t d", p=128)
            )
            nc.gpsimd.memset(vaug[:, :, D : D + 1], 1.0)

            # ---- xbar transposes: qt2/kt2 [128, M, 128] interleaved ----
            # One xbar per tensor: logical row r = t*64+d of Q^T lands on
            # partition (t*64+d)%128 = d + 64*(t%2), slot r//128 = t//2 = m:
            # qt2[0:64, m, j] = Q^T[d, tile 2m, col j]  (q = j*T + 2m)
            # qt2[64:128, m, j] = Q^T[d, tile 2m+1, col j]
            qt2 = tp.tile([128, M, 128], F16, tag="qt2")
            kt2 = tp.tile([128, M, 128], F16, tag="kt2")
            kt2s = tp.tile([128, M, 128], F16, tag="kt2s")
            nc.sync.dma_start(out=qt2, in_=q16, transpose=True)
            nc.sync.dma_start(out=kt2, in_=k16, transpose=True)
            # partition-swapped copy for the parity cross terms
            nc.gpsimd.dma_start(out=kt2s[0:64], in_=kt2[64:128])
            nc.gpsimd.dma_start(out=kt2s[64:128], in_=kt2[0:64])

            qt2f = qt2.rearrange("p m j -> p (m j)")
            kt2f = kt2.rearrange("p m j -> p (m j)")
            kt2sf = kt2s.rearrange("p m j -> p (m j)")

            # ---- QK -> exp -> PV (PV lagged one step for pipeline slack) ----
            # poT bank j = 2*half + c accumulates [O; denom]^T [65, 512] for
            # q-columns (parity=half, 512-chunk c) over all 16 k-tiles.
            poT = pp_o.tile([65, 4, 512], F32, tag="poT", name=f"poT{bh}")
            pending = []
            for s in range(2 * M + 1):
                if s < 2 * M:
                    m, cross = s // 2, s % 2
                    kkf = kt2sf if cross else kt2f
                    tiles = []
                    for half in (0, 1):
                        kb = 2 * m + (cross if half == 0 else 1 - cross)
                        lo, hi = 64 * half, 64 * (half + 1)
                        for c2 in (0, 1):
                            ps = pp_s.tile(
                                [128, 512], F32, tag="ps",
                                name=f"ps{bh}_{s}_{half}_{c2}",
                            )
                            nc.tensor.matmul(
                                out=ps,
                                lhsT=kkf[lo:hi, 128 * m : 128 * (m + 1)],
                                rhs=qt2f[lo:hi, 512 * c2 : 512 * (c2 + 1)],
                                start=True,
                                stop=True,
                            )
                            e = ep.tile([128, 512], F16, tag="e")
                            ti = (s * 4 + half * 2 + c2) % 16
                            if ti in (1, 3, 6, 8, 10, 12, 14):  # 7/16 on DVE
                                nc.vector.tensor_scalar(
                                    out=e.bitcast(I16),
                                    in0=ps,
                                    scalar1=KSC,
                                    scalar2=KOFF,
                                    op0=MULT,
                                    op1=ADD,
                                )
                            else:  # 9/16 exact on ScalarE
                                nc.scalar.activation(out=e, in_=ps, func=EXP, scale=SC)
                            tiles.append((e, kb, half, c2))
                    pending.append((s, tiles))
                if s >= 1:
                    sp, tiles = pending.pop(0)
                    for e, kb, half, c2 in tiles:
                        nc.tensor.matmul(
                            out=poT[:, 2 * half + c2, :],
                            lhsT=vaug[:, kb, :],
                            rhs=e,
                            start=sp == 0,
                            stop=sp == 2 * M - 1,
                        )

            # ---- epilogue ----
            # Assemble out^T [65, 2048] fp16 with natural q columns:
            # poT[:, 2*half+c, mq*128 + j] corresponds to q = j*16 + 8*c +
            # 2*mq + half.
            # outT col C = t*128 + j holds q = j*16 + t, so the xbar (which
            # writes logical row r to partition r%128, free slot r//128)
            # lands q exactly at [p=q//16, t=q%16]. Padded to 80 partitions
            # (xbar needs %16) so the denom row rides the same transpose.
            outT = ot.tile([80, S], F16, tag="outT")
            outT_r = outT[0:65].rearrange("p (t j) -> p t j", t=16)
            for j4 in range(4):
                half, c = j4 // 2, j4 % 2
                base = 8 * c + half
                nc.vector.tensor_copy(
                    out=outT_r[:, base : base + 7 : 2, :],
                    in_=poT[:, j4, :].rearrange("p (mq j) -> p mq j", mq=4),
                )
            out16x = of.tile([128, T, 80], F16, tag="out16x")
            nc.sync.dma_start(out=out16x, in_=outT, transpose=True)
            rcp = of.tile([128, T], F32, tag="rcp")
            nc.vector.reciprocal(out=rcp, in_=out16x[:, :, 64])
            outf = of.tile([128, T, D], F32, tag="outf")
            for t in range(T):
                nc.vector.tensor_scalar_mul(
                    out=outf[:, t, :], in0=out16x[:, t, 0:D], scalar1=rcp[:, t : t + 1]
                )
            nc.gpsimd.dma_start(
                out=o[bh].rearrange("(p t) d -> p t d", p=128), in_=outf
            )
    nc.compile()
    return nc


@functools.lru_cache(maxsize=1)
def _built():
    return build_attention()


def run(query, key, value, trace=False):
    """Shard (b,h) pairs 8 ways, run on cores 0-7, gather. Returns
    (out [B,H,S,D] f32, BassKernelResults)."""
    nc = _built()
    qf = np.ascontiguousarray(np.asarray(query, dtype=np.float32).reshape(B * H, S, D))
    kf = np.ascontiguousarray(np.asarray(key, dtype=np.float32).reshape(B * H, S, D))
    vf = np.ascontiguousarray(np.asarray(value, dtype=np.float32).reshape(B * H, S, D))
    in_maps = []
    for c in range(N_CORES):
        sl = slice(c * NBH, (c + 1) * NBH)
        in_maps.append(
            {
                "q": np.ascontiguousarray(qf[sl]),
                "k": np.ascontiguousarray(kf[sl]),
                "v": np.ascontiguousarray(vf[sl]),
            }
        )
    res = None
    last_err = None
    for attempt in range(3):
        try:
            res = run_bass_kernel_spmd(
                nc, in_maps, core_ids=list(range(N_CORES)), trace=trace
            )
            break
        except Exception as e:  # transient device wedge: retry
            last_err = e
            import time as _time

            _time.sleep(5 * (attempt + 1))
    if res is None:
        raise last_err
    out = np.concatenate([res.results[c]["o"] for c in range(N_CORES)], axis=0)
    return out.reshape(B, H, S, D).astype(np.float32), res


def kernel(query, key, value):
    out, _ = run(query, key, value)
    return out
